# revision 1
# baseline (speedup 1.0000x reference)
"""DilatedRNNStack Trainium2 kernel.

Strategy: data-parallel over batch (B=512 -> 64 rows/core on 8 cores).
Feature-major layout on-chip: every activation tile is [features(part), batch(free)].
Gate matmuls: g.T[128,64] = lhsT.T @ rhs with W stationary, batch on the free dim.

State feature order is permuted host-side to h-first ([orig 96:128, orig 0:96]) so:
  - h slices live at partitions 0:32 -> written straight into ring-buffer tiles
  - o slices live at partitions 32:128 -> written into `whole` tiles whose rows
    0:32 are zero; the next layer consumes whole[0:128] as a K=128 matmul chunk
    against weights whose first 32 rows are zero. No partition-shift copies.
Biases ride as an extra constant-1.0 row on the h-delay ring chunk (K=33).

Time loop is a software-pipelined wavefront: at tick s, layer l works on t=s-l,
so each cross-layer edge has one full tick of slack. Rings are indexed t%d which
is static inside a 12-step-unrolled For_i body (12 = lcm of dilations 1,3,6,12).
"""

import numpy as np

T, B, BSH = 512, 512, 64
DIL = (1, 3, 6, 12)
NCHUNK = 43  # ceil(515/12): x/y staged in 12-step chunks of [64, 12*64]

_prog = None


def _build():
    global _prog
    if _prog is not None:
        return _prog
    import concourse.bass as bass
    import concourse.tile as tile
    from concourse import bacc, mybir

    f32 = mybir.dt.float32
    Tanh = mybir.ActivationFunctionType.Tanh
    Sig = mybir.ActivationFunctionType.Sigmoid

    nc = bacc.Bacc("TRN2", target_bir_lowering=False, debug=False, num_devices=8)
    x_ap = nc.dram_tensor("x", [NCHUNK, 64, 768], f32, kind="ExternalInput").ap()
    y_ap = nc.dram_tensor("y", [NCHUNK, 64, 768], f32, kind="ExternalOutput").ap()
    wA_ap = [nc.dram_tensor(f"wA{l}", [64 if l == 0 else 128, 512], f32,
                            kind="ExternalInput").ap() for l in range(4)]
    wB20_ap = nc.dram_tensor("wB20", [33, 512], f32, kind="ExternalInput").ap()
    wP_ap = [None] + [nc.dram_tensor(f"wP{l}", [65, 512], f32,
                                     kind="ExternalInput").ap() for l in (1, 2, 3)]
    wY_ap = nc.dram_tensor("wY", [128, 64], f32, kind="ExternalInput").ap()

    with tile.TileContext(nc) as tc:
        import contextlib
        ctx = contextlib.ExitStack()
        with ctx:
            wpool = ctx.enter_context(tc.tile_pool(name="w", bufs=1))
            state = ctx.enter_context(tc.tile_pool(name="state", bufs=1))
            xin = ctx.enter_context(tc.tile_pool(name="xin", bufs=3))
            gpool = ctx.enter_context(tc.tile_pool(name="gates", bufs=8))
            tpool = ctx.enter_context(tc.tile_pool(name="tmp", bufs=24))
            spool = ctx.enter_context(tc.tile_pool(name="stage", bufs=2))
            ypool = ctx.enter_context(tc.tile_pool(name="yout", bufs=3))
            pgate = ctx.enter_context(tc.tile_pool(name="psg", bufs=6, space="PSUM"))
            py = ctx.enter_context(tc.tile_pool(name="psy", bufs=1, space="PSUM"))

            # --- load weights ---
            wA = [wpool.tile([64 if l == 0 else 128, 512], f32, name=f"wA{l}", tag=f"wA{l}")
                  for l in range(4)]
            wB20 = wpool.tile([33, 512], f32, name="wB20", tag="wB20")
            wP = [None] + [wpool.tile([65, 512], f32, name=f"wP{l}", tag=f"wP{l}") for l in (1, 2, 3)]
            wY = wpool.tile([128, 64], f32, name="wY", tag="wY")
            nc.sync.dma_start(wB20, wB20_ap)
            for l in range(4):
                nc.sync.dma_start(wA[l], wA_ap[l])
                if l > 0:
                    nc.sync.dma_start(wP[l], wP_ap[l])
            nc.sync.dma_start(wY, wY_ap)

            # --- persistent state ---
            # h rings: rows 0:32 = h, row 32 = 1.0 (bias row). c rings [128, 64*d].
            hr = [state.tile([33, 64 * d], f32, name=f"hr{l}", tag=f"hr{l}") for l, d in enumerate(DIL)]
            cr = [state.tile([128, 64 * d], f32, name=f"cr{l}", tag=f"cr{l}") for l, d in enumerate(DIL)]
            # whole tiles (layers 0..2 feed next layer / shortcut), triple-buffered
            whole = [[state.tile([128, 64], f32, name=f"wh{l}_{p}", tag=f"wh{l}_{p}") for p in range(3)]
                     for l in range(3)]
            for l in range(4):
                nc.vector.memset(hr[l][0:32, :], 0.0)
                nc.vector.memset(hr[l][32:33, :], 1.0)
                nc.vector.memset(cr[l], 0.0)
            for l in range(3):
                for p in range(3):
                    nc.vector.memset(whole[l][p][0:32, :], 0.0)
            # pair tiles for layers 1..3: rows 0:32 = h(t-1), 32:64 = h(t-d), 64 = 1.0
            pp = [None] + [[state.tile([65, 64], f32, name=f"pp{l}_{p}", tag=f"pp{l}_{p}")
                            for p in range(2)] for l in (1, 2, 3)]
            for l in (1, 2, 3):
                for p in range(2):
                    nc.vector.memset(pp[l][p][0:32, :], 0.0)
                    nc.vector.memset(pp[l][p][32:64, :], 0.0)
                    nc.vector.memset(pp[l][p][64:65, :], 1.0)

            def tmp():
                return tpool.tile([128, 64], f32, name="ct", tag="ct")

            def cell(l, t, u, xt, st):
                """Emit layer-l cell for step t; u = tick%12 (x slot / stage slot)."""
                d = DIL[l]
                sc = (t % d) * 64
                sp = ((t - 1) % d) * 64
                sdel = sc if t >= d else sp
                ps = pgate.tile([128, 256], f32, name="ps", tag="ps")
                for gi in range(4):
                    o = ps[:, 64 * gi:64 * gi + 64]
                    gsl = slice(128 * gi, 128 * gi + 128)
                    if l == 0:
                        nc.tensor.matmul(o, wA[0][:, gsl], xt[:, 64 * u:64 * u + 64],
                                         start=True, stop=False)
                        nc.tensor.matmul(o, wB20[:, gsl], hr[0][0:33, 0:64],
                                         start=False, stop=True)
                    else:
                        nc.tensor.matmul(o, wA[l][:, gsl], whole[l - 1][t % 3][:, :],
                                         start=True, stop=False)
                        nc.tensor.matmul(o, wP[l][:, gsl], pp[l][t % 2][:, :],
                                         start=False, stop=True)
                g = gpool.tile([128, 256], f32, name="g", tag="g")
                cslot = cr[l][:, sc:sc + 64]
                if t == 0:
                    nc.scalar.activation(cslot, ps[:, 0:64], Tanh)
                    nc.scalar.activation(g[:, 64:256], ps[:, 64:256], Sig)
                else:
                    nc.scalar.activation(g[:, 0:64], ps[:, 0:64], Tanh)
                    nc.scalar.activation(g[:, 64:256], ps[:, 64:256], Sig)
                    cand = g[:, 0:64]
                    f_ = g[:, 64:128]
                    al = g[:, 128:192]
                    pc = cr[l][:, sp:sp + 64]
                    if t >= d and d > 1:
                        dc = cr[l][:, sc:sc + 64]
                        t1 = tmp()
                        nc.vector.tensor_sub(t1, pc, dc)
                        t2 = tmp()
                        nc.vector.tensor_mul(t2, al, t1)
                        t3 = tmp()
                        nc.vector.tensor_add(t3, t2, dc)  # weighted
                        t4 = tmp()
                        nc.vector.tensor_sub(t4, t3, cand)
                    else:
                        t4 = tmp()
                        nc.vector.tensor_sub(t4, pc, cand)
                    t5 = tmp()
                    nc.vector.tensor_mul(t5, f_, t4)
                    nc.vector.tensor_add(cslot, t5, cand)  # new_c -> ring
                # wh: h part (partitions 0:32) -> h ring slot
                eng_wh = nc.vector if l == 0 else nc.gpsimd
                eng_wh.tensor_mul(hr[l][0:32, sc:sc + 64], g[0:32, 192:256],
                                  cr[l][0:32, sc:sc + 64])
                if l > 0:
                    # assemble pair tile for step t+1: prev = h(t) (plain copy),
                    # del = h(t+1-d) (partition-shift SBUF->SBUF DMA, slack d-1 steps)
                    t1 = t + 1
                    np_ = pp[l][t1 % 2]
                    sdn = ((t1 % d) if t1 >= d else ((t1 - 1) % d)) * 64
                    nc.gpsimd.tensor_copy(np_[0:32, :], hr[l][0:32, sc:sc + 64])
                    nc.sync.dma_start(np_[32:64, :], hr[l][0:32, sdn:sdn + 64])
                # wo: o part (partitions 32:128)
                if l == 3:
                    wo3 = tmp()
                    for lo, hi in ((32, 64), (64, 128)):
                        nc.vector.tensor_mul(wo3[lo:hi, :], g[lo:hi, 192:256],
                                             cr[l][lo:hi, sc:sc + 64])
                        nc.vector.tensor_add(st[lo:hi, 64 * u:64 * u + 64],
                                             wo3[lo:hi, :],
                                             whole[1][t % 3][lo:hi, :])
                else:
                    for lo, hi in ((32, 64), (64, 128)):
                        nc.vector.tensor_mul(whole[l][t % 3][lo:hi, :],
                                             g[lo:hi, 192:256],
                                             cr[l][lo:hi, sc:sc + 64])

            def tick(s, u, xt, st):
                for l in range(4):
                    t = s - l
                    if 0 <= t <= T - 1:
                        cell(l, t, u, xt, st)

            def emit_y(st, ncols):
                psy = py.tile([64, 768], f32, name="psy", tag="psy")
                nc.tensor.matmul(psy[:, 0:512], wY, st[:, 0:512], start=True, stop=True)
                if ncols > 512:
                    nc.tensor.matmul(psy[:, 512:768], wY, st[:, 512:768],
                                     start=True, stop=True)
                yt = ypool.tile([64, 768], f32, name="yt", tag="yt")
                nc.scalar.copy(yt[:, 0:ncols], psy[:, 0:ncols])
                return yt

            def new_stage(full_zero):
                st = spool.tile([128, 768], f32, name="st", tag="st")
                nc.vector.memset(st[0:32, :], 0.0)
                nc.vector.memset(st[0:1, :], 1.0)  # bias row (after zeroing 0:32)
                if full_zero:
                    nc.vector.memset(st[32:64, :], 0.0)
                    nc.vector.memset(st[64:128, :], 0.0)
                return st

            # ---- prologue: ticks 0..23 (chunks 0 and 1, static) ----
            for ch in range(2):
                xt = xin.tile([64, 768], f32, name="xt", tag="xt")
                nc.sync.dma_start(xt, x_ap[ch:ch + 1])
                st = new_stage(full_zero=(ch == 0))
                for u in range(12):
                    tick(12 * ch + u, u, xt, st)
                yt = emit_y(st, 768)
                nc.sync.dma_start(y_ap[ch:ch + 1], yt)

            # ---- steady loop: ticks 24..503 (chunks 2..41) ----
            # Inside the body only tick%12 == u is known; all ring/parity mods
            # use a steady representative t = 48+u-l, which matches the real
            # t = 12*i+u-l mod every d (12 % d == 0) and mod 3 (48 % 3 == 0).
            with tc.For_i(2, 42) as iv:
                xt = xin.tile([64, 768], f32, name="xt", tag="xt")
                nc.sync.dma_start(xt, x_ap[bass.ds(iv, 1)])
                st = new_stage(full_zero=False)
                for u in range(12):
                    for l in range(4):
                        cell(l, 48 + u - l, u, xt, st)
                yt = emit_y(st, 768)
                nc.sync.dma_start(y_ap[bass.ds(iv, 1)], yt)

            # ---- epilogue: ticks 504..514 (chunk 42) ----
            xt = xin.tile([64, 768], f32, name="xt", tag="xt")
            nc.sync.dma_start(xt, x_ap[42:43])
            st = new_stage(full_zero=True)
            for u in range(11):
                tick(504 + u, u, xt, st)
            yt = emit_y(st, 768)
            nc.sync.dma_start(y_ap[42:43], yt)

    nc.compile()
    _prog = nc
    return nc


def _prep_weights(ws, bs, Wa, ba):
    PERM = np.r_[96:128, 0:96]
    GORD = [1, 0, 2, 3]  # psum order: cand, forget(+1), alpha, outgate
    ins = [64, 96, 96, 96]
    out = {}
    for l in range(4):
        W, b = ws[l], bs[l]
        Wg = W.reshape(4, 128, -1)[GORD][:, PERM, :]  # [4,128,fan]
        bg = b.reshape(4, 128)[GORD][:, PERM].copy()
        bg[1] += 1.0
        n = ins[l]
        if l == 0:
            A = np.zeros((64, 512), np.float32)
            B2 = np.zeros((33, 512), np.float32)
            for gi in range(4):
                A[:, 128 * gi:128 * gi + 128] = Wg[gi, :, 0:64].T
                B2[0:32, 128 * gi:128 * gi + 128] = (
                    Wg[gi, :, 64:96] + Wg[gi, :, 96:128]).T
                B2[32, 128 * gi:128 * gi + 128] = bg[gi]
            out["wA0"], out["wB20"] = A, B2
        else:
            A = np.zeros((128, 512), np.float32)
            P = np.zeros((65, 512), np.float32)
            for gi in range(4):
                A[32:128, 128 * gi:128 * gi + 128] = Wg[gi, :, 0:96].T
                P[0:32, 128 * gi:128 * gi + 128] = Wg[gi, :, 96:128].T
                P[32:64, 128 * gi:128 * gi + 128] = Wg[gi, :, 128:160].T
                P[64, 128 * gi:128 * gi + 128] = bg[gi]
            out[f"wA{l}"], out[f"wP{l}"] = A, P
    WY = np.zeros((128, 64), np.float32)
    WY[0] = ba
    WY[32:128] = Wa.T
    out["wY"] = WY
    return out


def _run(inputs, trace=False):
    from concourse.bass_utils import run_bass_kernel_spmd

    x = np.ascontiguousarray(np.asarray(inputs["x"], dtype=np.float32))
    ws = [np.asarray(inputs[f"W{l}"], np.float32) for l in range(4)]
    bs = [np.asarray(inputs[f"b{l}"], np.float32) for l in range(4)]
    Wa = np.asarray(inputs["Wa"], np.float32)
    ba = np.asarray(inputs["ba"], np.float32)

    wmap = _prep_weights(ws, bs, Wa, ba)
    nc = _build()

    in_maps = []
    for c in range(8):
        xc = x[:, BSH * c:BSH * c + BSH, :].transpose(0, 2, 1)  # [512, 64f, 64b]
        xp = np.concatenate([xc, np.zeros((NCHUNK * 12 - T, 64, 64), np.float32)])
        xdev = np.ascontiguousarray(
            xp.reshape(NCHUNK, 12, 64, 64).transpose(0, 2, 1, 3).reshape(NCHUNK, 64, 768))
        in_maps.append({"x": xdev, **wmap})

    res = run_bass_kernel_spmd(nc, in_maps, list(range(8)), trace=trace)

    y = np.empty((T, B, 64), np.float32)
    for c in range(8):
        ydev = res.results[c]["y"]  # [43, 64, 768]
        z = ydev.reshape(NCHUNK, 64, 12, 64).transpose(0, 2, 3, 1).reshape(NCHUNK * 12, 64, 64)
        y[:, BSH * c:BSH * c + BSH, :] = z[3:3 + T]  # skew: y(t) at tick t+3
    return y, res


def _time_exec(nc, in_maps, iters=20):
    """Steady-state wall-clock of the compiled NEFF via a reusable jitted fn."""
    import time
    import jax
    import jax.numpy as jnp
    from jax.sharding import Mesh, PartitionSpec
    from jax.experimental.shard_map import shard_map
    from concourse import bass2jax, mybir

    bass2jax.install_neuronx_cc_hook()
    n_cores = len(in_maps)
    partition_name = nc.partition_id_tensor.name if nc.partition_id_tensor else None
    in_names, out_names, out_avals, zero_outs = [], [], [], []
    for alloc in nc.m.functions[0].allocations:
        if not isinstance(alloc, mybir.MemoryLocationSet):
            continue
        name = alloc.memorylocations[0].name
        if alloc.kind == "ExternalInput":
            if name != partition_name:
                in_names.append(name)
        elif alloc.kind == "ExternalOutput":
            shape = list(alloc.tensor_shape)
            npdt = mybir.dt.np(alloc.dtype)
            out_avals.append(jax.core.ShapedArray(shape, npdt))
            out_names.append(name)
            zero_outs.append(np.zeros(shape, npdt))

    n_params = len(in_names)
    n_outs = len(out_names)
    all_in_names = in_names + out_names
    if partition_name is not None:
        all_in_names = all_in_names + [partition_name]
    donate = tuple(range(n_params, n_params + n_outs))

    def _body(*args):
        operands = list(args)
        if partition_name is not None:
            operands.append(bass2jax.partition_id_tensor())
        return tuple(bass2jax._bass_exec_p.bind(
            *operands, out_avals=tuple(out_avals), in_names=tuple(all_in_names),
            out_names=tuple(out_names), lowering_input_output_aliases=(),
            sim_require_finite=True, sim_require_nnan=True, nc=nc))

    devices = jax.devices()[:n_cores]
    mesh = Mesh(np.asarray(devices), ("core",))
    nin = n_params + n_outs
    sharded = jax.jit(shard_map(
        _body, mesh=mesh, in_specs=(PartitionSpec("core"),) * nin,
        out_specs=(PartitionSpec("core"),) * n_outs, check_rep=False),
        donate_argnums=donate, keep_unused=True)
    concat_in = [np.concatenate([m[name] for m in in_maps], axis=0)
                 for name in in_names]
    concat_zeros = [np.zeros((n_cores * z.shape[0], *z.shape[1:]), z.dtype)
                    for z in zero_outs]
    in_args = [jax.device_put(a) for a in concat_in]
    zouts = [jax.device_put(a) for a in concat_zeros]
    out = sharded(*in_args, *zouts)
    jax.block_until_ready(out)
    times = []
    for _ in range(iters):
        # recycle outputs as the donated out-buffers (kernel writes all of y)
        t0 = time.perf_counter()
        out = sharded(*in_args, *list(out))
        jax.block_until_ready(out)
        times.append(time.perf_counter() - t0)
    return min(times), times


def kernel(**inputs):
    y, _ = _run(inputs, trace=False)
    return y



# revision 11
# speedup vs baseline: 4.9951x; 4.9951x over previous
"""DilatedRNNStack Trainium2 kernel.

Strategy: data-parallel over batch (B=512 -> 64 rows/core on 8 cores).
Feature-major layout on-chip: every activation tile is [features(part), batch(free)].
Gate matmuls: g.T[128,64] = lhsT.T @ rhs with W stationary, batch on the free dim.

State feature order is permuted host-side to h-first ([orig 96:128, orig 0:96]) so:
  - h slices live at partitions 0:32 -> written straight into ring-buffer tiles
  - o slices live at partitions 32:128 -> written into `whole` tiles whose rows
    0:32 are zero; the next layer consumes whole[0:128] as a K=128 matmul chunk
    against weights whose first 32 rows are zero. No partition-shift copies.
Biases ride as an extra constant-1.0 row on the h ring (K=33).
Recurrent gate inputs h(t-1) and h(t-d) are consumed DIRECTLY from the ring
tiles as two extra K-chunks per gate (wH1/wH2), so there is no per-cell pair
tile assembly and no SBUF->SBUF partition-shift DMA in the time loop.

Time loop is a software-pipelined wavefront: at tick s, layer l works on t=s-l,
so each cross-layer edge has one full tick of slack. Rings are indexed t%d which
is static inside a 12-step-unrolled For_i body (12 = lcm of dilations 1,3,6,12).
"""

import numpy as np

T, B, BSH = 512, 512, 64
DIL = (1, 3, 6, 12)
NCHUNK = 43  # ceil(515/12): x/y staged in 12-step chunks of [64, 12*64]

_prog = None


def _build(unroll=False):
    """unroll=True replaces the For_i hardware loop with a python loop
    (identical per-iteration instruction stream) — used only by profiling
    scripts; the graded path uses the default."""
    global _prog
    if _prog is not None and not unroll:
        return _prog
    import concourse.bass as bass
    import concourse.tile as tile
    from concourse import bacc, mybir

    f32 = mybir.dt.float32
    Tanh = mybir.ActivationFunctionType.Tanh
    Sig = mybir.ActivationFunctionType.Sigmoid

    nc = bacc.Bacc("TRN2", target_bir_lowering=False, debug=False, num_devices=8)
    x_ap = nc.dram_tensor("x", [NCHUNK, 64, 768], f32, kind="ExternalInput").ap()
    y_ap = nc.dram_tensor("y", [NCHUNK, 64, 768], f32, kind="ExternalOutput").ap()
    wA_ap = [nc.dram_tensor(f"wA{l}", [64 if l == 0 else 128, 512], f32,
                            kind="ExternalInput").ap() for l in range(4)]
    wB20_ap = nc.dram_tensor("wB20", [33, 512], f32, kind="ExternalInput").ap()
    wH1_ap = [None] + [nc.dram_tensor(f"wH1{l}", [33, 512], f32,
                                      kind="ExternalInput").ap() for l in (1, 2, 3)]
    wH2_ap = [None] + [nc.dram_tensor(f"wH2{l}", [32, 512], f32,
                                      kind="ExternalInput").ap() for l in (1, 2, 3)]
    wY_ap = nc.dram_tensor("wY", [128, 64], f32, kind="ExternalInput").ap()

    with tile.TileContext(nc) as tc:
        import contextlib
        ctx = contextlib.ExitStack()
        with ctx:
            wpool = ctx.enter_context(tc.tile_pool(name="w", bufs=1))
            state = ctx.enter_context(tc.tile_pool(name="state", bufs=1))
            xin = ctx.enter_context(tc.tile_pool(name="xin", bufs=3))
            gpool = ctx.enter_context(tc.tile_pool(name="gates", bufs=8))
            tpool = ctx.enter_context(tc.tile_pool(name="tmp", bufs=24))
            spool = ctx.enter_context(tc.tile_pool(name="stage", bufs=2))
            ypool = ctx.enter_context(tc.tile_pool(name="yout", bufs=3))
            pgate = ctx.enter_context(tc.tile_pool(name="psg", bufs=6, space="PSUM"))
            py = ctx.enter_context(tc.tile_pool(name="psy", bufs=1, space="PSUM"))

            # --- load weights ---
            wA = [wpool.tile([64 if l == 0 else 128, 512], f32, name=f"wA{l}", tag=f"wA{l}")
                  for l in range(4)]
            wB20 = wpool.tile([33, 512], f32, name="wB20", tag="wB20")
            wH1 = [None] + [wpool.tile([33, 512], f32, name=f"wH1{l}", tag=f"wH1{l}")
                            for l in (1, 2, 3)]
            wH2 = [None] + [wpool.tile([32, 512], f32, name=f"wH2{l}", tag=f"wH2{l}")
                            for l in (1, 2, 3)]
            wY = wpool.tile([128, 64], f32, name="wY", tag="wY")
            nc.sync.dma_start(wB20, wB20_ap)
            for l in range(4):
                nc.sync.dma_start(wA[l], wA_ap[l])
                if l > 0:
                    nc.sync.dma_start(wH1[l], wH1_ap[l])
                    nc.sync.dma_start(wH2[l], wH2_ap[l])
            nc.sync.dma_start(wY, wY_ap)

            # --- persistent state ---
            # h rings: rows 0:32 = h, row 32 = 1.0 (bias row). c rings [128, 64*d].
            hr = [state.tile([33, 64 * d], f32, name=f"hr{l}", tag=f"hr{l}") for l, d in enumerate(DIL)]
            cr = [state.tile([128, 64 * d], f32, name=f"cr{l}", tag=f"cr{l}") for l, d in enumerate(DIL)]
            # whole tiles (layers 0..2 feed next layer / shortcut), triple-buffered
            whole = [[state.tile([128, 64], f32, name=f"wh{l}_{p}", tag=f"wh{l}_{p}") for p in range(3)]
                     for l in range(3)]
            for l in range(4):
                nc.vector.memset(hr[l][0:32, :], 0.0)
                nc.vector.memset(hr[l][32:33, :], 1.0)
                nc.vector.memset(cr[l], 0.0)
            for l in range(3):
                for p in range(3):
                    nc.vector.memset(whole[l][p][0:32, :], 0.0)

            def tmp():
                return tpool.tile([128, 64], f32, name="ct", tag="ct")

            def cell(l, t, u, xt, st):
                """Emit layer-l cell for step t; u = tick%12 (x slot / stage slot)."""
                d = DIL[l]
                sc = (t % d) * 64
                sp = ((t - 1) % d) * 64
                sdel = sc if t >= d else sp
                ps = pgate.tile([128, 256], f32, name="ps", tag="ps")
                for gi in range(4):
                    o = ps[:, 64 * gi:64 * gi + 64]
                    gsl = slice(128 * gi, 128 * gi + 128)
                    if l == 0:
                        nc.tensor.matmul(o, wA[0][:, gsl], xt[:, 64 * u:64 * u + 64],
                                         start=True, stop=False)
                        nc.tensor.matmul(o, wB20[:, gsl], hr[0][0:33, 0:64],
                                         start=False, stop=True)
                    else:
                        nc.tensor.matmul(o, wA[l][:, gsl], whole[l - 1][t % 3][:, :],
                                         start=True, stop=False)
                        nc.tensor.matmul(o, wH1[l][:, gsl], hr[l][0:33, sp:sp + 64],
                                         start=False, stop=False)
                        nc.tensor.matmul(o, wH2[l][:, gsl], hr[l][0:32, sdel:sdel + 64],
                                         start=False, stop=True)
                g = gpool.tile([128, 256], f32, name="g", tag="g")
                cslot = cr[l][:, sc:sc + 64]
                if t == 0:
                    nc.scalar.activation(cslot, ps[:, 0:64], Tanh)
                    nc.scalar.activation(g[:, 64:256], ps[:, 64:256], Sig)
                else:
                    nc.scalar.activation(g[:, 0:64], ps[:, 0:64], Tanh)
                    nc.scalar.activation(g[:, 64:256], ps[:, 64:256], Sig)
                    cand = g[:, 0:64]
                    f_ = g[:, 64:128]
                    al = g[:, 128:192]
                    pc = cr[l][:, sp:sp + 64]
                    if t >= d and d > 1:
                        dc = cr[l][:, sc:sc + 64]
                        t1 = tmp()
                        nc.vector.tensor_sub(t1, pc, dc)
                        t2 = tmp()
                        nc.vector.tensor_mul(t2, al, t1)
                        t3 = tmp()
                        nc.vector.tensor_add(t3, t2, dc)  # weighted
                        t4 = tmp()
                        nc.vector.tensor_sub(t4, t3, cand)
                    else:
                        t4 = tmp()
                        nc.vector.tensor_sub(t4, pc, cand)
                    t5 = tmp()
                    nc.vector.tensor_mul(t5, f_, t4)
                    nc.vector.tensor_add(cslot, t5, cand)  # new_c -> ring
                # wh: h part (partitions 0:32) -> h ring slot
                eng_wh = nc.vector if l == 0 else nc.gpsimd
                eng_wh.tensor_mul(hr[l][0:32, sc:sc + 64], g[0:32, 192:256],
                                  cr[l][0:32, sc:sc + 64])
                # wo: o part (partitions 32:128)
                if l == 3:
                    wo3 = tmp()
                    for lo, hi in ((32, 64), (64, 128)):
                        nc.vector.tensor_mul(wo3[lo:hi, :], g[lo:hi, 192:256],
                                             cr[l][lo:hi, sc:sc + 64])
                        nc.vector.tensor_add(st[lo:hi, 64 * u:64 * u + 64],
                                             wo3[lo:hi, :],
                                             whole[1][t % 3][lo:hi, :])
                else:
                    for lo, hi in ((32, 64), (64, 128)):
                        nc.vector.tensor_mul(whole[l][t % 3][lo:hi, :],
                                             g[lo:hi, 192:256],
                                             cr[l][lo:hi, sc:sc + 64])

            def tick(s, u, xt, st):
                for l in range(4):
                    t = s - l
                    if 0 <= t <= T - 1:
                        cell(l, t, u, xt, st)

            def emit_y(st, ncols):
                psy = py.tile([64, 768], f32, name="psy", tag="psy")
                nc.tensor.matmul(psy[:, 0:512], wY, st[:, 0:512], start=True, stop=True)
                if ncols > 512:
                    nc.tensor.matmul(psy[:, 512:768], wY, st[:, 512:768],
                                     start=True, stop=True)
                yt = ypool.tile([64, 768], f32, name="yt", tag="yt")
                nc.scalar.copy(yt[:, 0:ncols], psy[:, 0:ncols])
                return yt

            def new_stage(full_zero):
                st = spool.tile([128, 768], f32, name="st", tag="st")
                nc.vector.memset(st[0:32, :], 0.0)
                nc.vector.memset(st[0:1, :], 1.0)  # bias row (after zeroing 0:32)
                if full_zero:
                    nc.vector.memset(st[32:64, :], 0.0)
                    nc.vector.memset(st[64:128, :], 0.0)
                return st

            # ---- prologue: ticks 0..23 (chunks 0 and 1, static) ----
            for ch in range(2):
                xt = xin.tile([64, 768], f32, name="xt", tag="xt")
                nc.sync.dma_start(xt, x_ap[ch:ch + 1])
                st = new_stage(full_zero=(ch == 0))
                for u in range(12):
                    tick(12 * ch + u, u, xt, st)
                yt = emit_y(st, 768)
                nc.sync.dma_start(y_ap[ch:ch + 1], yt)

            # ---- steady loop: ticks 24..503 (chunks 2..41) ----
            # Inside the body only tick%12 == u is known; all ring/parity mods
            # use a steady representative t = 48+u-l, which matches the real
            # t = 12*i+u-l mod every d (12 % d == 0) and mod 3 (48 % 3 == 0).
            import contextlib as _cl

            def loop_ctx():
                return (_cl.nullcontext(enumerate(range(2, 42))) if unroll
                        else tc.For_i(2, 42))

            with loop_ctx() as iv_:
                for iv in ([iv_] if not unroll else [v for _, v in iv_]):
                    xt = xin.tile([64, 768], f32, name="xt", tag="xt")
                    nc.sync.dma_start(xt, x_ap[bass.ds(iv, 1)])
                    st = new_stage(full_zero=False)
                    for u in range(12):
                        for l in range(4):
                            cell(l, 48 + u - l, u, xt, st)
                    yt = emit_y(st, 768)
                    nc.sync.dma_start(y_ap[bass.ds(iv, 1)], yt)

            # ---- epilogue: ticks 504..514 (chunk 42) ----
            xt = xin.tile([64, 768], f32, name="xt", tag="xt")
            nc.sync.dma_start(xt, x_ap[42:43])
            st = new_stage(full_zero=True)
            for u in range(11):
                tick(504 + u, u, xt, st)
            yt = emit_y(st, 768)
            nc.sync.dma_start(y_ap[42:43], yt)

    nc.compile()
    _prog = nc
    return nc


def _prep_weights(ws, bs, Wa, ba):
    PERM = np.r_[96:128, 0:96]
    GORD = [1, 0, 2, 3]  # psum order: cand, forget(+1), alpha, outgate
    ins = [64, 96, 96, 96]
    out = {}
    for l in range(4):
        W, b = ws[l], bs[l]
        Wg = W.reshape(4, 128, -1)[GORD][:, PERM, :]  # [4,128,fan]
        bg = b.reshape(4, 128)[GORD][:, PERM].copy()
        bg[1] += 1.0
        n = ins[l]
        if l == 0:
            A = np.zeros((64, 512), np.float32)
            B2 = np.zeros((33, 512), np.float32)
            for gi in range(4):
                A[:, 128 * gi:128 * gi + 128] = Wg[gi, :, 0:64].T
                B2[0:32, 128 * gi:128 * gi + 128] = (
                    Wg[gi, :, 64:96] + Wg[gi, :, 96:128]).T
                B2[32, 128 * gi:128 * gi + 128] = bg[gi]
            out["wA0"], out["wB20"] = A, B2
        else:
            A = np.zeros((128, 512), np.float32)
            H1 = np.zeros((33, 512), np.float32)
            H2 = np.zeros((32, 512), np.float32)
            for gi in range(4):
                A[32:128, 128 * gi:128 * gi + 128] = Wg[gi, :, 0:96].T
                H1[0:32, 128 * gi:128 * gi + 128] = Wg[gi, :, 96:128].T
                H1[32, 128 * gi:128 * gi + 128] = bg[gi]
                H2[0:32, 128 * gi:128 * gi + 128] = Wg[gi, :, 128:160].T
            out[f"wA{l}"], out[f"wH1{l}"], out[f"wH2{l}"] = A, H1, H2
    WY = np.zeros((128, 64), np.float32)
    WY[0] = ba
    WY[32:128] = Wa.T
    out["wY"] = WY
    return out


def _run(inputs, trace=False):
    from concourse.bass_utils import run_bass_kernel_spmd

    x = np.ascontiguousarray(np.asarray(inputs["x"], dtype=np.float32))
    ws = [np.asarray(inputs[f"W{l}"], np.float32) for l in range(4)]
    bs = [np.asarray(inputs[f"b{l}"], np.float32) for l in range(4)]
    Wa = np.asarray(inputs["Wa"], np.float32)
    ba = np.asarray(inputs["ba"], np.float32)

    wmap = _prep_weights(ws, bs, Wa, ba)
    nc = _build()

    in_maps = []
    for c in range(8):
        xc = x[:, BSH * c:BSH * c + BSH, :].transpose(0, 2, 1)  # [512, 64f, 64b]
        xp = np.concatenate([xc, np.zeros((NCHUNK * 12 - T, 64, 64), np.float32)])
        xdev = np.ascontiguousarray(
            xp.reshape(NCHUNK, 12, 64, 64).transpose(0, 2, 1, 3).reshape(NCHUNK, 64, 768))
        in_maps.append({"x": xdev, **wmap})

    res = run_bass_kernel_spmd(nc, in_maps, list(range(8)), trace=trace)

    y = np.empty((T, B, 64), np.float32)
    for c in range(8):
        ydev = res.results[c]["y"]  # [43, 64, 768]
        z = ydev.reshape(NCHUNK, 64, 12, 64).transpose(0, 2, 3, 1).reshape(NCHUNK * 12, 64, 64)
        y[:, BSH * c:BSH * c + BSH, :] = z[3:3 + T]  # skew: y(t) at tick t+3
    return y, res


def _time_exec(nc, in_maps, iters=20):
    """Steady-state wall-clock of the compiled NEFF via a reusable jitted fn."""
    import time
    import jax
    import jax.numpy as jnp
    from jax.sharding import Mesh, PartitionSpec
    from jax.experimental.shard_map import shard_map
    from concourse import bass2jax, mybir

    bass2jax.install_neuronx_cc_hook()
    n_cores = len(in_maps)
    partition_name = nc.partition_id_tensor.name if nc.partition_id_tensor else None
    in_names, out_names, out_avals, zero_outs = [], [], [], []
    for alloc in nc.m.functions[0].allocations:
        if not isinstance(alloc, mybir.MemoryLocationSet):
            continue
        name = alloc.memorylocations[0].name
        if alloc.kind == "ExternalInput":
            if name != partition_name:
                in_names.append(name)
        elif alloc.kind == "ExternalOutput":
            shape = list(alloc.tensor_shape)
            npdt = mybir.dt.np(alloc.dtype)
            out_avals.append(jax.core.ShapedArray(shape, npdt))
            out_names.append(name)
            zero_outs.append(np.zeros(shape, npdt))

    n_params = len(in_names)
    n_outs = len(out_names)
    all_in_names = in_names + out_names
    if partition_name is not None:
        all_in_names = all_in_names + [partition_name]
    donate = tuple(range(n_params, n_params + n_outs))

    def _body(*args):
        operands = list(args)
        if partition_name is not None:
            operands.append(bass2jax.partition_id_tensor())
        return tuple(bass2jax._bass_exec_p.bind(
            *operands, out_avals=tuple(out_avals), in_names=tuple(all_in_names),
            out_names=tuple(out_names), lowering_input_output_aliases=(),
            sim_require_finite=True, sim_require_nnan=True, nc=nc))

    devices = jax.devices()[:n_cores]
    mesh = Mesh(np.asarray(devices), ("core",))
    nin = n_params + n_outs
    sharded = jax.jit(shard_map(
        _body, mesh=mesh, in_specs=(PartitionSpec("core"),) * nin,
        out_specs=(PartitionSpec("core"),) * n_outs, check_rep=False),
        donate_argnums=donate, keep_unused=True)
    concat_in = [np.concatenate([m[name] for m in in_maps], axis=0)
                 for name in in_names]
    concat_zeros = [np.zeros((n_cores * z.shape[0], *z.shape[1:]), z.dtype)
                    for z in zero_outs]
    in_args = [jax.device_put(a) for a in concat_in]
    zouts = [jax.device_put(a) for a in concat_zeros]
    out = sharded(*in_args, *zouts)
    jax.block_until_ready(out)
    # Sustained execution rate: enqueue `iters` back-to-back executions
    # (device-serialized via the donated output buffers) and block once.
    # A blocking wall-clock per call would be dominated by the ~70-100 ms
    # axon tunnel round-trip, masking the NEFF time entirely.
    for _ in range(10):  # warm the dispatch pipeline
        out = sharded(*in_args, *list(out))
    jax.block_until_ready(out)
    times = []
    for _ in range(3):
        t0 = time.perf_counter()
        for _ in range(iters):
            out = sharded(*in_args, *list(out))
        jax.block_until_ready(out)
        times.append((time.perf_counter() - t0) / iters)
    return min(times), times


def kernel(**inputs):
    y, _ = _run(inputs, trace=False)
    return y



# revision 14
# speedup vs baseline: 6.1041x; 1.2220x over previous
"""DilatedRNNStack Trainium2 kernel.

Strategy: data-parallel over batch (B=512 -> 64 rows/core on 8 cores).
Feature-major layout on-chip: every activation tile is [features(part), batch(free)].
Gate matmuls: g.T[128,64] = lhsT.T @ rhs with W stationary, batch on the free dim.

State feature order is permuted host-side to h-first ([orig 96:128, orig 0:96]) so:
  - h slices live at partitions 0:32 -> written straight into ring-buffer tiles
  - o slices live at partitions 32:128 -> written into `whole` tiles whose rows
    0:32 are zero; the next layer consumes whole[0:128] as a K=128 matmul chunk
    against weights whose first 32 rows are zero. No partition-shift copies.
Biases ride as an extra constant-1.0 row on the h ring (K=33).
Recurrent gate inputs h(t-1) and h(t-d) are consumed DIRECTLY from the ring
tiles as two extra K-chunks per gate (wH1/wH2), so there is no per-cell pair
tile assembly and no SBUF->SBUF partition-shift DMA in the time loop.

Time loop is a software-pipelined wavefront: at tick s, layer l works on t=s-l,
so each cross-layer edge has one full tick of slack. Rings are indexed t%d which
is static inside a 12-step-unrolled For_i body (12 = lcm of dilations 1,3,6,12).
"""

import numpy as np

T, B, BSH = 512, 512, 64
DIL = (1, 3, 6, 12)
NCHUNK = 43  # ceil(515/12): x/y staged in 12-step chunks of [64, 12*64]

_prog = None


def _build(unroll=False):
    """unroll=True replaces the For_i hardware loop with a python loop
    (identical per-iteration instruction stream) — used only by profiling
    scripts; the graded path uses the default."""
    global _prog
    if _prog is not None and not unroll:
        return _prog
    import concourse.bass as bass
    import concourse.tile as tile
    from concourse import bacc, mybir

    f32 = mybir.dt.float32
    f16 = mybir.dt.float16
    Tanh = mybir.ActivationFunctionType.Tanh
    Sig = mybir.ActivationFunctionType.Sigmoid

    nc = bacc.Bacc("TRN2", target_bir_lowering=False, debug=False, num_devices=8)
    x_ap = nc.dram_tensor("x", [NCHUNK, 64, 768], f16, kind="ExternalInput").ap()
    y_ap = nc.dram_tensor("y", [NCHUNK, 64, 768], f32, kind="ExternalOutput").ap()
    wA_ap = [nc.dram_tensor(f"wA{l}", [64 if l == 0 else 128, 512], f16,
                            kind="ExternalInput").ap() for l in range(4)]
    wB20_ap = nc.dram_tensor("wB20", [33, 512], f16, kind="ExternalInput").ap()
    wH1_ap = [None] + [nc.dram_tensor(f"wH1{l}", [33, 512], f16,
                                      kind="ExternalInput").ap() for l in (1, 2, 3)]
    wH2_ap = [None] + [nc.dram_tensor(f"wH2{l}", [32, 512], f16,
                                      kind="ExternalInput").ap() for l in (1, 2, 3)]
    wY_ap = nc.dram_tensor("wY", [128, 64], f16, kind="ExternalInput").ap()

    with tile.TileContext(nc) as tc:
        import contextlib
        ctx = contextlib.ExitStack()
        with ctx:
            wpool = ctx.enter_context(tc.tile_pool(name="w", bufs=1))
            state = ctx.enter_context(tc.tile_pool(name="state", bufs=1))
            xin = ctx.enter_context(tc.tile_pool(name="xin", bufs=3))
            gpool = ctx.enter_context(tc.tile_pool(name="gates", bufs=8))
            tpool = ctx.enter_context(tc.tile_pool(name="tmp", bufs=24))
            spool = ctx.enter_context(tc.tile_pool(name="stage", bufs=2))
            ypool = ctx.enter_context(tc.tile_pool(name="yout", bufs=3))
            pgate = ctx.enter_context(tc.tile_pool(name="psg", bufs=6, space="PSUM"))
            py = ctx.enter_context(tc.tile_pool(name="psy", bufs=1, space="PSUM"))

            # --- load weights ---
            wA = [wpool.tile([64 if l == 0 else 128, 512], f16, name=f"wA{l}", tag=f"wA{l}")
                  for l in range(4)]
            wB20 = wpool.tile([33, 512], f16, name="wB20", tag="wB20")
            wH1 = [None] + [wpool.tile([33, 512], f16, name=f"wH1{l}", tag=f"wH1{l}")
                            for l in (1, 2, 3)]
            wH2 = [None] + [wpool.tile([32, 512], f16, name=f"wH2{l}", tag=f"wH2{l}")
                            for l in (1, 2, 3)]
            wY = wpool.tile([128, 64], f16, name="wY", tag="wY")
            nc.sync.dma_start(wB20, wB20_ap)
            for l in range(4):
                nc.sync.dma_start(wA[l], wA_ap[l])
                if l > 0:
                    nc.sync.dma_start(wH1[l], wH1_ap[l])
                    nc.sync.dma_start(wH2[l], wH2_ap[l])
            nc.sync.dma_start(wY, wY_ap)

            # --- persistent state ---
            # h rings: rows 0:32 = h, row 32 = 1.0 (bias row). c rings [128, 64*d].
            hr = [state.tile([33, 64 * d], f16, name=f"hr{l}", tag=f"hr{l}") for l, d in enumerate(DIL)]
            cr = [state.tile([128, 64 * d], f32, name=f"cr{l}", tag=f"cr{l}") for l, d in enumerate(DIL)]
            # whole tiles (layers 0..2 feed next layer / shortcut), triple-buffered
            whole = [[state.tile([128, 64], f16, name=f"wh{l}_{p}", tag=f"wh{l}_{p}") for p in range(3)]
                     for l in range(3)]
            for l in range(4):
                nc.vector.memset(hr[l][0:32, :], 0.0)
                nc.vector.memset(hr[l][32:33, :], 1.0)
                nc.vector.memset(cr[l], 0.0)
            for l in range(3):
                for p in range(3):
                    nc.vector.memset(whole[l][p][0:32, :], 0.0)

            def tmp():
                return tpool.tile([128, 64], f32, name="ct", tag="ct")

            def cell(l, t, u, xt, st):
                """Emit layer-l cell for step t; u = tick%12 (x slot / stage slot)."""
                d = DIL[l]
                sc = (t % d) * 64
                sp = ((t - 1) % d) * 64
                sdel = sc if t >= d else sp
                ps = pgate.tile([128, 256], f32, name="ps", tag="ps")
                for gi in range(4):
                    o = ps[:, 64 * gi:64 * gi + 64]
                    gsl = slice(128 * gi, 128 * gi + 128)
                    if l == 0:
                        nc.tensor.matmul(o, wA[0][:, gsl], xt[:, 64 * u:64 * u + 64],
                                         start=True, stop=False)
                        nc.tensor.matmul(o, wB20[:, gsl], hr[0][0:33, 0:64],
                                         start=False, stop=True)
                    else:
                        nc.tensor.matmul(o, wA[l][:, gsl], whole[l - 1][t % 3][:, :],
                                         start=True, stop=False)
                        nc.tensor.matmul(o, wH1[l][:, gsl], hr[l][0:33, sp:sp + 64],
                                         start=False, stop=False)
                        nc.tensor.matmul(o, wH2[l][:, gsl], hr[l][0:32, sdel:sdel + 64],
                                         start=False, stop=True)
                g = gpool.tile([128, 256], f32, name="g", tag="g")
                cslot = cr[l][:, sc:sc + 64]
                if t == 0:
                    nc.scalar.activation(cslot, ps[:, 0:64], Tanh)
                    nc.scalar.activation(g[:, 64:256], ps[:, 64:256], Sig)
                else:
                    nc.scalar.activation(g[:, 0:64], ps[:, 0:64], Tanh)
                    nc.scalar.activation(g[:, 64:256], ps[:, 64:256], Sig)
                    cand = g[:, 0:64]
                    f_ = g[:, 64:128]
                    al = g[:, 128:192]
                    pc = cr[l][:, sp:sp + 64]
                    if t >= d and d > 1:
                        dc = cr[l][:, sc:sc + 64]
                        t1 = tmp()
                        nc.vector.tensor_sub(t1, pc, dc)
                        t2 = tmp()
                        nc.vector.tensor_mul(t2, al, t1)
                        t3 = tmp()
                        nc.vector.tensor_add(t3, t2, dc)  # weighted
                        t4 = tmp()
                        nc.vector.tensor_sub(t4, t3, cand)
                    else:
                        t4 = tmp()
                        nc.vector.tensor_sub(t4, pc, cand)
                    t5 = tmp()
                    nc.vector.tensor_mul(t5, f_, t4)
                    nc.vector.tensor_add(cslot, t5, cand)  # new_c -> ring
                # wh: h part (partitions 0:32) -> h ring slot
                eng_wh = nc.vector if l == 0 else nc.gpsimd
                eng_wh.tensor_mul(hr[l][0:32, sc:sc + 64], g[0:32, 192:256],
                                  cr[l][0:32, sc:sc + 64])
                # wo: o part (partitions 32:128)
                if l == 3:
                    wo3 = tmp()
                    for lo, hi in ((32, 64), (64, 128)):
                        nc.vector.tensor_mul(wo3[lo:hi, :], g[lo:hi, 192:256],
                                             cr[l][lo:hi, sc:sc + 64])
                        nc.vector.tensor_add(st[lo:hi, 64 * u:64 * u + 64],
                                             wo3[lo:hi, :],
                                             whole[1][t % 3][lo:hi, :])
                else:
                    for lo, hi in ((32, 64), (64, 128)):
                        nc.vector.tensor_mul(whole[l][t % 3][lo:hi, :],
                                             g[lo:hi, 192:256],
                                             cr[l][lo:hi, sc:sc + 64])

            def tick(s, u, xt, st):
                for l in range(4):
                    t = s - l
                    if 0 <= t <= T - 1:
                        cell(l, t, u, xt, st)

            def emit_y(st, ncols):
                psy = py.tile([64, 768], f32, name="psy", tag="psy")
                nc.tensor.matmul(psy[:, 0:512], wY, st[:, 0:512], start=True, stop=True)
                if ncols > 512:
                    nc.tensor.matmul(psy[:, 512:768], wY, st[:, 512:768],
                                     start=True, stop=True)
                yt = ypool.tile([64, 768], f32, name="yt", tag="yt")
                nc.scalar.copy(yt[:, 0:ncols], psy[:, 0:ncols])
                return yt

            def new_stage(full_zero):
                st = spool.tile([128, 768], f16, name="st", tag="st")
                nc.vector.memset(st[0:32, :], 0.0)
                nc.vector.memset(st[0:1, :], 1.0)  # bias row (after zeroing 0:32)
                if full_zero:
                    nc.vector.memset(st[32:64, :], 0.0)
                    nc.vector.memset(st[64:128, :], 0.0)
                return st

            # ---- prologue: ticks 0..23 (chunks 0 and 1, static) ----
            for ch in range(2):
                xt = xin.tile([64, 768], f16, name="xt", tag="xt")
                nc.sync.dma_start(xt, x_ap[ch:ch + 1])
                st = new_stage(full_zero=(ch == 0))
                for u in range(12):
                    tick(12 * ch + u, u, xt, st)
                yt = emit_y(st, 768)
                nc.sync.dma_start(y_ap[ch:ch + 1], yt)

            # ---- steady loop: ticks 24..503 (chunks 2..41) ----
            # Inside the body only tick%12 == u is known; all ring/parity mods
            # use a steady representative t = 48+u-l, which matches the real
            # t = 12*i+u-l mod every d (12 % d == 0) and mod 3 (48 % 3 == 0).
            import contextlib as _cl

            def loop_ctx():
                return (_cl.nullcontext(enumerate(range(2, 42))) if unroll
                        else tc.For_i(2, 42))

            with loop_ctx() as iv_:
                for iv in ([iv_] if not unroll else [v for _, v in iv_]):
                    xt = xin.tile([64, 768], f16, name="xt", tag="xt")
                    nc.sync.dma_start(xt, x_ap[bass.ds(iv, 1)])
                    st = new_stage(full_zero=False)
                    for u in range(12):
                        for l in range(4):
                            cell(l, 48 + u - l, u, xt, st)
                    yt = emit_y(st, 768)
                    nc.sync.dma_start(y_ap[bass.ds(iv, 1)], yt)

            # ---- epilogue: ticks 504..514 (chunk 42) ----
            xt = xin.tile([64, 768], f16, name="xt", tag="xt")
            nc.sync.dma_start(xt, x_ap[42:43])
            st = new_stage(full_zero=True)
            for u in range(11):
                tick(504 + u, u, xt, st)
            yt = emit_y(st, 768)
            nc.sync.dma_start(y_ap[42:43], yt)

    nc.compile()
    _prog = nc
    return nc


def _prep_weights(ws, bs, Wa, ba):
    PERM = np.r_[96:128, 0:96]
    GORD = [1, 0, 2, 3]  # psum order: cand, forget(+1), alpha, outgate
    ins = [64, 96, 96, 96]
    out = {}
    for l in range(4):
        W, b = ws[l], bs[l]
        Wg = W.reshape(4, 128, -1)[GORD][:, PERM, :]  # [4,128,fan]
        bg = b.reshape(4, 128)[GORD][:, PERM].copy()
        bg[1] += 1.0
        n = ins[l]
        if l == 0:
            A = np.zeros((64, 512), np.float32)
            B2 = np.zeros((33, 512), np.float32)
            for gi in range(4):
                A[:, 128 * gi:128 * gi + 128] = Wg[gi, :, 0:64].T
                B2[0:32, 128 * gi:128 * gi + 128] = (
                    Wg[gi, :, 64:96] + Wg[gi, :, 96:128]).T
                B2[32, 128 * gi:128 * gi + 128] = bg[gi]
            out["wA0"], out["wB20"] = A, B2
        else:
            A = np.zeros((128, 512), np.float32)
            H1 = np.zeros((33, 512), np.float32)
            H2 = np.zeros((32, 512), np.float32)
            for gi in range(4):
                A[32:128, 128 * gi:128 * gi + 128] = Wg[gi, :, 0:96].T
                H1[0:32, 128 * gi:128 * gi + 128] = Wg[gi, :, 96:128].T
                H1[32, 128 * gi:128 * gi + 128] = bg[gi]
                H2[0:32, 128 * gi:128 * gi + 128] = Wg[gi, :, 128:160].T
            out[f"wA{l}"], out[f"wH1{l}"], out[f"wH2{l}"] = A, H1, H2
    WY = np.zeros((128, 64), np.float32)
    WY[0] = ba
    WY[32:128] = Wa.T
    out["wY"] = WY
    return {k: v.astype(np.float16) for k, v in out.items()}


def _make_in_maps(inputs):
    """Stage full inputs into the 8 per-core input maps (x as fp16 chunks)."""
    x = np.ascontiguousarray(np.asarray(inputs["x"], dtype=np.float32))
    ws = [np.asarray(inputs[f"W{l}"], np.float32) for l in range(4)]
    bs = [np.asarray(inputs[f"b{l}"], np.float32) for l in range(4)]
    wmap = _prep_weights(ws, bs, np.asarray(inputs["Wa"], np.float32),
                         np.asarray(inputs["ba"], np.float32))
    in_maps = []
    for c in range(8):
        xc = x[:, BSH * c:BSH * c + BSH, :].transpose(0, 2, 1)  # [512, 64f, 64b]
        xp = np.concatenate([xc, np.zeros((NCHUNK * 12 - T, 64, 64), np.float32)])
        xdev = np.ascontiguousarray(
            xp.reshape(NCHUNK, 12, 64, 64).transpose(0, 2, 1, 3)
            .reshape(NCHUNK, 64, 768).astype(np.float16))
        in_maps.append({"x": xdev, **wmap})
    return in_maps


def _run(inputs, trace=False):
    from concourse.bass_utils import run_bass_kernel_spmd

    nc = _build()
    in_maps = _make_in_maps(inputs)
    res = run_bass_kernel_spmd(nc, in_maps, list(range(8)), trace=trace)

    y = np.empty((T, B, 64), np.float32)
    for c in range(8):
        ydev = res.results[c]["y"]  # [43, 64, 768]
        z = ydev.reshape(NCHUNK, 64, 12, 64).transpose(0, 2, 3, 1).reshape(NCHUNK * 12, 64, 64)
        y[:, BSH * c:BSH * c + BSH, :] = z[3:3 + T]  # skew: y(t) at tick t+3
    return y, res


def _time_exec(nc, in_maps, iters=20):
    """Steady-state wall-clock of the compiled NEFF via a reusable jitted fn."""
    import time
    import jax
    import jax.numpy as jnp
    from jax.sharding import Mesh, PartitionSpec
    from jax.experimental.shard_map import shard_map
    from concourse import bass2jax, mybir

    bass2jax.install_neuronx_cc_hook()
    n_cores = len(in_maps)
    partition_name = nc.partition_id_tensor.name if nc.partition_id_tensor else None
    in_names, out_names, out_avals, zero_outs = [], [], [], []
    for alloc in nc.m.functions[0].allocations:
        if not isinstance(alloc, mybir.MemoryLocationSet):
            continue
        name = alloc.memorylocations[0].name
        if alloc.kind == "ExternalInput":
            if name != partition_name:
                in_names.append(name)
        elif alloc.kind == "ExternalOutput":
            shape = list(alloc.tensor_shape)
            npdt = mybir.dt.np(alloc.dtype)
            out_avals.append(jax.core.ShapedArray(shape, npdt))
            out_names.append(name)
            zero_outs.append(np.zeros(shape, npdt))

    n_params = len(in_names)
    n_outs = len(out_names)
    all_in_names = in_names + out_names
    if partition_name is not None:
        all_in_names = all_in_names + [partition_name]
    donate = tuple(range(n_params, n_params + n_outs))

    def _body(*args):
        operands = list(args)
        if partition_name is not None:
            operands.append(bass2jax.partition_id_tensor())
        return tuple(bass2jax._bass_exec_p.bind(
            *operands, out_avals=tuple(out_avals), in_names=tuple(all_in_names),
            out_names=tuple(out_names), lowering_input_output_aliases=(),
            sim_require_finite=True, sim_require_nnan=True, nc=nc))

    devices = jax.devices()[:n_cores]
    mesh = Mesh(np.asarray(devices), ("core",))
    nin = n_params + n_outs
    sharded = jax.jit(shard_map(
        _body, mesh=mesh, in_specs=(PartitionSpec("core"),) * nin,
        out_specs=(PartitionSpec("core"),) * n_outs, check_rep=False),
        donate_argnums=donate, keep_unused=True)
    concat_in = [np.concatenate([m[name] for m in in_maps], axis=0)
                 for name in in_names]
    concat_zeros = [np.zeros((n_cores * z.shape[0], *z.shape[1:]), z.dtype)
                    for z in zero_outs]
    in_args = [jax.device_put(a) for a in concat_in]
    zouts = [jax.device_put(a) for a in concat_zeros]
    out = sharded(*in_args, *zouts)
    jax.block_until_ready(out)
    # Sustained execution rate: enqueue `iters` back-to-back executions
    # (device-serialized via the donated output buffers) and block once.
    # A blocking wall-clock per call would be dominated by the ~70-100 ms
    # axon tunnel round-trip, masking the NEFF time entirely.
    for _ in range(10):  # warm the dispatch pipeline
        out = sharded(*in_args, *list(out))
    jax.block_until_ready(out)
    times = []
    for _ in range(3):
        t0 = time.perf_counter()
        for _ in range(iters):
            out = sharded(*in_args, *list(out))
        jax.block_until_ready(out)
        times.append((time.perf_counter() - t0) / iters)
    return min(times), times


def kernel(**inputs):
    y, _ = _run(inputs, trace=False)
    return y



# revision 20
# speedup vs baseline: 6.8595x; 1.1237x over previous
"""DilatedRNNStack Trainium2 kernel.

Strategy: data-parallel over batch (B=512 -> 64 rows/core on 8 cores).
Feature-major layout on-chip: every activation tile is [features(part), batch(free)].
Gate matmuls: g.T[128,64] = lhsT.T @ rhs with W stationary, batch on the free dim.

State feature order is permuted host-side to h-first ([orig 96:128, orig 0:96]) so:
  - h slices live at partitions 0:32 -> written straight into ring-buffer tiles
  - o slices live at partitions 32:128 -> written into `whole` tiles whose rows
    0:32 are zero; the next layer consumes whole[0:128] as a K=128 matmul chunk
    against weights whose first 32 rows are zero. No partition-shift copies.
Biases ride as an extra constant-1.0 row on the h ring (K=33).
Recurrent gate inputs h(t-1) and h(t-d) are consumed DIRECTLY from the ring
tiles as two extra K-chunks per gate (wH1/wH2), so there is no per-cell pair
tile assembly and no SBUF->SBUF partition-shift DMA in the time loop.

Time loop is a software-pipelined wavefront: at tick s, layer l works on t=s-l,
so each cross-layer edge has one full tick of slack. Rings are indexed t%d which
is static inside a 12-step-unrolled For_i body (12 = lcm of dilations 1,3,6,12).
"""

import numpy as np

T, B, BSH = 512, 512, 64
DIL = (1, 3, 6, 12)
NCHUNK = 43  # ceil(515/12): x/y staged in 12-step chunks of [64, 12*64]

_prog = None


def _build(unroll=False):
    """unroll=True replaces the For_i hardware loop with a python loop
    (identical per-iteration instruction stream) — used only by profiling
    scripts; the graded path uses the default."""
    global _prog
    if _prog is not None and not unroll:
        return _prog
    import concourse.bass as bass
    import concourse.tile as tile
    from concourse import bacc, mybir

    f32 = mybir.dt.float32
    f16 = mybir.dt.float16
    Tanh = mybir.ActivationFunctionType.Tanh
    Sig = mybir.ActivationFunctionType.Sigmoid

    nc = bacc.Bacc("TRN2", target_bir_lowering=False, debug=False, num_devices=8)
    x_ap = nc.dram_tensor("x", [NCHUNK, 64, 768], f16, kind="ExternalInput").ap()
    y_ap = nc.dram_tensor("y", [NCHUNK, 64, 768], f32, kind="ExternalOutput").ap()
    wA_ap = [nc.dram_tensor(f"wA{l}", [64 if l == 0 else 128, 512], f16,
                            kind="ExternalInput").ap() for l in range(4)]
    wB20_ap = nc.dram_tensor("wB20", [33, 512], f16, kind="ExternalInput").ap()
    wHD_ap = [None] + [nc.dram_tensor(f"wHD{l}", [33, 512], f16,
                                      kind="ExternalInput").ap() for l in (1, 2, 3)]
    wY_ap = nc.dram_tensor("wY", [128, 64], f16, kind="ExternalInput").ap()

    with tile.TileContext(nc) as tc:
        import contextlib
        ctx = contextlib.ExitStack()
        with ctx:
            wpool = ctx.enter_context(tc.tile_pool(name="w", bufs=1))
            state = ctx.enter_context(tc.tile_pool(name="state", bufs=1))
            xin = ctx.enter_context(tc.tile_pool(name="xin", bufs=3))
            gpool = ctx.enter_context(tc.tile_pool(name="gates", bufs=8))
            tpool = ctx.enter_context(tc.tile_pool(name="tmp", bufs=24))
            spool = ctx.enter_context(tc.tile_pool(name="stage", bufs=2))
            ypool = ctx.enter_context(tc.tile_pool(name="yout", bufs=3))
            pgate = ctx.enter_context(tc.tile_pool(name="psg", bufs=6, space="PSUM"))
            py = ctx.enter_context(tc.tile_pool(name="psy", bufs=1, space="PSUM"))

            # --- load weights ---
            wA = [wpool.tile([64 if l == 0 else 128, 512], f16, name=f"wA{l}", tag=f"wA{l}")
                  for l in range(4)]
            wB20 = wpool.tile([33, 512], f16, name="wB20", tag="wB20")
            wHD = [None] + [wpool.tile([33, 512], f16, name=f"wHD{l}", tag=f"wHD{l}")
                            for l in (1, 2, 3)]
            wY = wpool.tile([128, 64], f16, name="wY", tag="wY")
            nc.sync.dma_start(wB20, wB20_ap)
            for l in range(4):
                nc.sync.dma_start(wA[l], wA_ap[l])
                if l > 0:
                    nc.sync.dma_start(wHD[l], wHD_ap[l])
            nc.sync.dma_start(wY, wY_ap)

            # --- persistent state ---
            # h rings: rows 0:32 = h, row 32 = 1.0 (bias row). c rings [128, 64*d].
            hr = [state.tile([33, 64 * d], f16, name=f"hr{l}", tag=f"hr{l}") for l, d in enumerate(DIL)]
            cr = [state.tile([128, 64 * d], f32, name=f"cr{l}", tag=f"cr{l}") for l, d in enumerate(DIL)]
            # whole tiles (layers 0..2 feed next layer / shortcut), triple-buffered
            whole = [[state.tile([128, 64], f16, name=f"wh{l}_{p}", tag=f"wh{l}_{p}") for p in range(3)]
                     for l in range(3)]
            for l in range(4):
                nc.vector.memset(hr[l][0:32, :], 0.0)
                nc.vector.memset(hr[l][32:33, :], 1.0)
                nc.vector.memset(cr[l], 0.0)
            for l in range(3):
                for p in range(3):
                    nc.vector.memset(whole[l][p][0:32, :], 0.0)

            def tmp():
                return tpool.tile([128, 64], f32, name="ct", tag="ct")

            def cell(l, t, u, xt, st):
                """Emit layer-l cell for step t; u = tick%12 (x slot / stage slot)."""
                d = DIL[l]
                sc = (t % d) * 64
                sp = ((t - 1) % d) * 64
                sdel = sc if t >= d else sp
                ps = pgate.tile([128, 256], f32, name="ps", tag="ps")
                for gi in range(4):
                    o = ps[:, 64 * gi:64 * gi + 64]
                    gsl = slice(128 * gi, 128 * gi + 128)
                    if l == 0:
                        nc.tensor.matmul(o, wA[0][:, gsl], xt[:, 64 * u:64 * u + 64],
                                         start=True, stop=False)
                        nc.tensor.matmul(o, wB20[:, gsl], hr[0][0:33, 0:64],
                                         start=False, stop=True)
                    else:
                        # whole[l-1] rows 0:32 carry h_l(t-1); wA rows 0:32 are
                        # the prev-h weights, so one K=128 chunk covers inp+prev_h.
                        nc.tensor.matmul(o, wA[l][:, gsl], whole[l - 1][t % 3][:, :],
                                         start=True, stop=False)
                        nc.tensor.matmul(o, wHD[l][:, gsl], hr[l][0:33, sdel:sdel + 64],
                                         start=False, stop=True)
                g = gpool.tile([128, 256], f32, name="g", tag="g")
                cslot = cr[l][:, sc:sc + 64]
                if t == 0:
                    nc.scalar.activation(cslot, ps[:, 0:64], Tanh)
                    nc.scalar.activation(g[:, 64:256], ps[:, 64:256], Sig)
                else:
                    nc.scalar.activation(g[:, 0:64], ps[:, 0:64], Tanh)
                    nc.scalar.activation(g[:, 64:256], ps[:, 64:256], Sig)
                    cand = g[:, 0:64]
                    f_ = g[:, 64:128]
                    al = g[:, 128:192]
                    pc = cr[l][:, sp:sp + 64]
                    if t >= d and d > 1:
                        dc = cr[l][:, sc:sc + 64]
                        t1 = tmp()
                        nc.vector.tensor_sub(t1, pc, dc)
                        t2 = tmp()
                        nc.vector.tensor_mul(t2, al, t1)
                        t3 = tmp()
                        nc.vector.tensor_add(t3, t2, dc)  # weighted
                        t4 = tmp()
                        nc.vector.tensor_sub(t4, t3, cand)
                    else:
                        t4 = tmp()
                        nc.vector.tensor_sub(t4, pc, cand)
                    t5 = tmp()
                    nc.vector.tensor_mul(t5, f_, t4)
                    nc.vector.tensor_add(cslot, t5, cand)  # new_c -> ring
                # wh: h part (partitions 0:32) -> h ring slot
                eng_wh = nc.vector if l == 0 else nc.gpsimd
                eng_wh.tensor_mul(hr[l][0:32, sc:sc + 64], g[0:32, 192:256],
                                  cr[l][0:32, sc:sc + 64])
                if l > 0 and t + 1 <= T - 1:
                    # h_l(t) also rides rows 0:32 of whole[l-1] for step t+1,
                    # folding the prev-h matmul chunk into the K=128 wA chunk
                    nc.scalar.copy(whole[l - 1][(t + 1) % 3][0:32, :],
                                   hr[l][0:32, sc:sc + 64])
                # wo: o part (partitions 32:128; APs must stay in naturally
                # aligned partition blocks, so 32:64 and 64:128 are separate)
                if l == 3:
                    wo3 = tmp()
                    for lo, hi in ((32, 64), (64, 128)):
                        nc.vector.tensor_mul(wo3[lo:hi, :], g[lo:hi, 192:256],
                                             cr[l][lo:hi, sc:sc + 64])
                        nc.vector.tensor_add(st[lo:hi, 64 * u:64 * u + 64],
                                             wo3[lo:hi, :],
                                             whole[1][t % 3][lo:hi, :])
                else:
                    for lo, hi in ((32, 64), (64, 128)):
                        nc.vector.tensor_mul(whole[l][t % 3][lo:hi, :],
                                             g[lo:hi, 192:256],
                                             cr[l][lo:hi, sc:sc + 64])

            def tick(s, u, xt, st):
                for l in range(4):
                    t = s - l
                    if 0 <= t <= T - 1:
                        cell(l, t, u, xt, st)

            def emit_y(st, ncols):
                psy = py.tile([64, 768], f32, name="psy", tag="psy")
                nc.tensor.matmul(psy[:, 0:512], wY, st[:, 0:512], start=True, stop=True)
                if ncols > 512:
                    nc.tensor.matmul(psy[:, 512:768], wY, st[:, 512:768],
                                     start=True, stop=True)
                yt = ypool.tile([64, 768], f32, name="yt", tag="yt")
                nc.scalar.copy(yt[:, 0:ncols], psy[:, 0:ncols])
                return yt

            def new_stage(full_zero):
                st = spool.tile([128, 768], f16, name="st", tag="st")
                nc.vector.memset(st[0:32, :], 0.0)
                nc.vector.memset(st[0:1, :], 1.0)  # bias row (after zeroing 0:32)
                if full_zero:
                    nc.vector.memset(st[32:64, :], 0.0)
                    nc.vector.memset(st[64:128, :], 0.0)
                return st

            # ---- prologue: ticks 0..23 (chunks 0 and 1, static) ----
            for ch in range(2):
                xt = xin.tile([64, 768], f16, name="xt", tag="xt")
                nc.sync.dma_start(xt, x_ap[ch:ch + 1])
                st = new_stage(full_zero=(ch == 0))
                for u in range(12):
                    tick(12 * ch + u, u, xt, st)
                yt = emit_y(st, 768)
                nc.sync.dma_start(y_ap[ch:ch + 1], yt)

            # ---- steady loop: ticks 24..503 (chunks 2..41) ----
            # Inside the body only tick%12 == u is known; all ring/parity mods
            # use a steady representative t = 48+u-l, which matches the real
            # t = 12*i+u-l mod every d (12 % d == 0) and mod 3 (48 % 3 == 0).
            import contextlib as _cl

            def loop_ctx():
                return (_cl.nullcontext(enumerate(range(2, 42))) if unroll
                        else tc.For_i(2, 42))

            with loop_ctx() as iv_:
                for iv in ([iv_] if not unroll else [v for _, v in iv_]):
                    xt = xin.tile([64, 768], f16, name="xt", tag="xt")
                    nc.sync.dma_start(xt, x_ap[bass.ds(iv, 1)])
                    st = new_stage(full_zero=False)
                    for u in range(12):
                        for l in range(4):
                            cell(l, 48 + u - l, u, xt, st)
                    yt = emit_y(st, 768)
                    nc.sync.dma_start(y_ap[bass.ds(iv, 1)], yt)

            # ---- epilogue: ticks 504..514 (chunk 42) ----
            xt = xin.tile([64, 768], f16, name="xt", tag="xt")
            nc.sync.dma_start(xt, x_ap[42:43])
            st = new_stage(full_zero=True)
            for u in range(11):
                tick(504 + u, u, xt, st)
            yt = emit_y(st, 768)
            nc.sync.dma_start(y_ap[42:43], yt)

    nc.compile()
    _prog = nc
    return nc


def _prep_weights(ws, bs, Wa, ba):
    PERM = np.r_[96:128, 0:96]
    GORD = [1, 0, 2, 3]  # psum order: cand, forget(+1), alpha, outgate
    ins = [64, 96, 96, 96]
    out = {}
    for l in range(4):
        W, b = ws[l], bs[l]
        Wg = W.reshape(4, 128, -1)[GORD][:, PERM, :]  # [4,128,fan]
        bg = b.reshape(4, 128)[GORD][:, PERM].copy()
        bg[1] += 1.0
        n = ins[l]
        if l == 0:
            A = np.zeros((64, 512), np.float32)
            B2 = np.zeros((33, 512), np.float32)
            for gi in range(4):
                A[:, 128 * gi:128 * gi + 128] = Wg[gi, :, 0:64].T
                B2[0:32, 128 * gi:128 * gi + 128] = (
                    Wg[gi, :, 64:96] + Wg[gi, :, 96:128]).T
                B2[32, 128 * gi:128 * gi + 128] = bg[gi]
            out["wA0"], out["wB20"] = A, B2
        else:
            A = np.zeros((128, 512), np.float32)
            HD = np.zeros((33, 512), np.float32)
            for gi in range(4):
                A[0:32, 128 * gi:128 * gi + 128] = Wg[gi, :, 96:128].T
                A[32:128, 128 * gi:128 * gi + 128] = Wg[gi, :, 0:96].T
                HD[0:32, 128 * gi:128 * gi + 128] = Wg[gi, :, 128:160].T
                HD[32, 128 * gi:128 * gi + 128] = bg[gi]
            out[f"wA{l}"], out[f"wHD{l}"] = A, HD
    WY = np.zeros((128, 64), np.float32)
    WY[0] = ba
    WY[32:128] = Wa.T
    out["wY"] = WY
    return {k: v.astype(np.float16) for k, v in out.items()}


def _make_in_maps(inputs):
    """Stage full inputs into the 8 per-core input maps (x as fp16 chunks)."""
    x = np.ascontiguousarray(np.asarray(inputs["x"], dtype=np.float32))
    ws = [np.asarray(inputs[f"W{l}"], np.float32) for l in range(4)]
    bs = [np.asarray(inputs[f"b{l}"], np.float32) for l in range(4)]
    wmap = _prep_weights(ws, bs, np.asarray(inputs["Wa"], np.float32),
                         np.asarray(inputs["ba"], np.float32))
    in_maps = []
    for c in range(8):
        xc = x[:, BSH * c:BSH * c + BSH, :].transpose(0, 2, 1)  # [512, 64f, 64b]
        xp = np.concatenate([xc, np.zeros((NCHUNK * 12 - T, 64, 64), np.float32)])
        xdev = np.ascontiguousarray(
            xp.reshape(NCHUNK, 12, 64, 64).transpose(0, 2, 1, 3)
            .reshape(NCHUNK, 64, 768).astype(np.float16))
        in_maps.append({"x": xdev, **wmap})
    return in_maps


def _run(inputs, trace=False):
    from concourse.bass_utils import run_bass_kernel_spmd

    nc = _build()
    in_maps = _make_in_maps(inputs)
    res = run_bass_kernel_spmd(nc, in_maps, list(range(8)), trace=trace)

    y = np.empty((T, B, 64), np.float32)
    for c in range(8):
        ydev = res.results[c]["y"]  # [43, 64, 768]
        z = ydev.reshape(NCHUNK, 64, 12, 64).transpose(0, 2, 3, 1).reshape(NCHUNK * 12, 64, 64)
        y[:, BSH * c:BSH * c + BSH, :] = z[3:3 + T]  # skew: y(t) at tick t+3
    return y, res


def _time_exec(nc, in_maps, iters=20):
    """Steady-state wall-clock of the compiled NEFF via a reusable jitted fn."""
    import time
    import jax
    import jax.numpy as jnp
    from jax.sharding import Mesh, PartitionSpec
    from jax.experimental.shard_map import shard_map
    from concourse import bass2jax, mybir

    bass2jax.install_neuronx_cc_hook()
    n_cores = len(in_maps)
    partition_name = nc.partition_id_tensor.name if nc.partition_id_tensor else None
    in_names, out_names, out_avals, zero_outs = [], [], [], []
    for alloc in nc.m.functions[0].allocations:
        if not isinstance(alloc, mybir.MemoryLocationSet):
            continue
        name = alloc.memorylocations[0].name
        if alloc.kind == "ExternalInput":
            if name != partition_name:
                in_names.append(name)
        elif alloc.kind == "ExternalOutput":
            shape = list(alloc.tensor_shape)
            npdt = mybir.dt.np(alloc.dtype)
            out_avals.append(jax.core.ShapedArray(shape, npdt))
            out_names.append(name)
            zero_outs.append(np.zeros(shape, npdt))

    n_params = len(in_names)
    n_outs = len(out_names)
    all_in_names = in_names + out_names
    if partition_name is not None:
        all_in_names = all_in_names + [partition_name]
    donate = tuple(range(n_params, n_params + n_outs))

    def _body(*args):
        operands = list(args)
        if partition_name is not None:
            operands.append(bass2jax.partition_id_tensor())
        return tuple(bass2jax._bass_exec_p.bind(
            *operands, out_avals=tuple(out_avals), in_names=tuple(all_in_names),
            out_names=tuple(out_names), lowering_input_output_aliases=(),
            sim_require_finite=True, sim_require_nnan=True, nc=nc))

    devices = jax.devices()[:n_cores]
    mesh = Mesh(np.asarray(devices), ("core",))
    nin = n_params + n_outs
    sharded = jax.jit(shard_map(
        _body, mesh=mesh, in_specs=(PartitionSpec("core"),) * nin,
        out_specs=(PartitionSpec("core"),) * n_outs, check_rep=False),
        donate_argnums=donate, keep_unused=True)
    concat_in = [np.concatenate([m[name] for m in in_maps], axis=0)
                 for name in in_names]
    concat_zeros = [np.zeros((n_cores * z.shape[0], *z.shape[1:]), z.dtype)
                    for z in zero_outs]
    in_args = [jax.device_put(a) for a in concat_in]
    zouts = [jax.device_put(a) for a in concat_zeros]
    out = sharded(*in_args, *zouts)
    jax.block_until_ready(out)
    # Sustained execution rate: enqueue `iters` back-to-back executions
    # (device-serialized via the donated output buffers) and block once.
    # A blocking wall-clock per call would be dominated by the ~70-100 ms
    # axon tunnel round-trip, masking the NEFF time entirely.
    for _ in range(10):  # warm the dispatch pipeline
        out = sharded(*in_args, *list(out))
    jax.block_until_ready(out)
    times = []
    for _ in range(3):
        t0 = time.perf_counter()
        for _ in range(iters):
            out = sharded(*in_args, *list(out))
        jax.block_until_ready(out)
        times.append((time.perf_counter() - t0) / iters)
    return min(times), times


def kernel(**inputs):
    y, _ = _run(inputs, trace=False)
    return y

